# revision 9
# baseline (speedup 1.0000x reference)
"""ExtractOverlappingPatches Trainium2 kernel, v11.

Input  x:   (16, 64, 128, 128) f32
Output y:   (16, 576, 128, 128) f32 where
            y[b, c*9 + (i*3+j), h, w] = x[b, c, h+i-1, w+j-1] (zero padded).

Strategy (pure memory movement, target_regime=memory):
  - Shard batch 16 -> 2 per core across 8 NeuronCores; run the same
    single-core program everywhere (data-parallel over batch).
  - Device input is a guard-padded flat layout: one zero row above and below
    the (b c h) = q row stack, plus one zero guard column appended to every
    row (row pitch 129) and one leading zero element.  Every out-of-range
    column read of a shift then lands on a zero guard element, so horizontal
    boundaries come out correct straight from the copies.
  - Device output is f-major [9, PB, C, H, W]: for fixed f the output block
    is a flat image stack, so a shift is a strided flat copy.  The three
    shifts sharing an i are ONE 3-dim DMA [q=16384, f=3, w=128] whose middle
    dim walks both the output f blocks and the source column offsets
    (source step 1 elem).  All DRAM->DRAM, no SBUF staging.
  - The only remaining defects are the h=0 / h=127 boundary rows of the
    i=0 / i=2 triples (the flat q axis wraps into the neighboring image);
    ONE fused DMA [384, 2, 128] overwrites all of them with zeros from a
    const tensor.  It is ordered behind both producer triples by explicit
    completion-semaphore waits before issue -- ring FIFO alone does NOT
    order byte-level write-after-write on real queues (descriptors of
    adjacent DMAs drain on different SDMA engines).  The i=1 triple has no
    boundary defects, so it runs concurrently with the fill on the other
    ring.
  - 4 DMAs over the two HWDGE rings (SP + ACT): the i=0 and i=2 triples at
    ring depth 1, then the fill next to the i=1 triple at depth 2.  The
    SWDGE (gpsimd) ring is not used: its descriptor ring tops out below the
    49k descriptors a triple needs.
  - Host gather transposes [9, PB, C, H, W] -> [PB, C*9, H, W] per core.
"""

import dataclasses

import numpy as np

import concourse.bass as bass
import concourse.mybir as mybir
from concourse.bass_utils import run_bass_kernel_spmd

N_CORES = 8
B, C, H, W = 16, 64, 128, 128
PB = B // N_CORES  # batches per core
KH, KW = 3, 3
F = KH * KW
P = PB * C  # images per core == 128
Q = P * H  # merged (b c h) rows per core == 16384
WP = W + 1  # padded row pitch (zero guard column)
XPAD = 1 + (Q + 2) * WP  # leading zero + (guard row, q rows, guard row)

_cache = {}


def _prep(x_shard: np.ndarray) -> np.ndarray:
    """Pack one core's [PB, C, H, W] input into the guard-padded flat layout."""
    buf = np.zeros(XPAD, dtype=np.float32)
    rows = buf[1 + WP : 1 + WP * (Q + 1)].reshape(Q, WP)
    rows[:, :W] = x_shard.reshape(Q, W)
    return buf


def _build() -> bass.Bass:
    nc = bass.Bass()
    dt = mybir.dt.float32
    x = nc.dram_tensor("x", [XPAD], dt, kind="ExternalInput")
    out = nc.dram_tensor("out", [F, PB, C, H, W], dt, kind="ExternalOutput")
    zeros = nc.inline_tensor(np.zeros(2 * KW * P * W, dtype=np.float32), name="zconst")

    # buf index of x[r, c] is 1 + (r+1)*WP + c; shift f=(i,j) at out row q
    # reads r = q+i-1, c = w+j-1  ->  src offset (i*WP + j) + q*WP + w.
    def copy_group(f0, n, f_step, src_step):
        """One DMA covering shifts f0, f0+f_step, ... (n of them)."""
        i0, j0 = divmod(f0, KW)
        o = dataclasses.replace(
            out[0, 0, 0, 0, :],
            offset=f0 * Q * W,
            ap=[[W, Q], [f_step * Q * W, n], [1, W]],
        )
        i_ = dataclasses.replace(
            x[:],
            offset=i0 * WP + j0,
            ap=[[WP, Q], [src_step, n], [1, W]],
        )
        return o, i_

    def row_fill(f0, n):
        """Zero h=0 rows of shifts f0..f0+n-1 (i=0) and h=127 rows of
        f0+6..f0+6+n-1 (i=2): dims [(f b c)=n*128, h-side pair=2, w=128]."""
        o = dataclasses.replace(
            out[0, 0, 0, 0, :],
            offset=f0 * Q * W,
            ap=[[H * W, n * P], [2 * KW * Q * W + (H - 1) * W, 2], [1, W]],
        )
        z = dataclasses.replace(
            zeros[:], offset=0, ap=[[W, n * P], [n * P * W, 2], [1, W]]
        )
        return o, z

    with (
        nc.semaphore("st") as st,  # ACT ring: i=2 triple completion
        nc.semaphore("sa") as sa,  # SP ring: i=0 triple completion
        nc.semaphore("sd") as sd,  # everything else / final drain
    ):
        with nc.Block() as block:
            # SP ring: i=0 triple, then the fused boundary-row fill (after
            # both producer triples' completion sems).  ACT ring: i=2
            # triple first (the fill waits on it), then the i=1 triple.
            @block.scalar
            def _(scalar):
                o, i_ = copy_group(6, 3, 1, 1)  # i=2 triple (f6,f7,f8)
                scalar.dma_start(out=o, in_=i_).then_inc(st, 16)
                o, i_ = copy_group(3, 3, 1, 1)  # i=1 triple (f3,f4,f5)
                scalar.dma_start(out=o, in_=i_).then_inc(sd, 16)

            @block.sync
            def _(sync):
                o, i_ = copy_group(0, 3, 1, 1)  # i=0 triple (f0,f1,f2)
                sync.dma_start(out=o, in_=i_).then_inc(sa, 16)
                sync.wait_ge(sa, 16)
                sync.wait_ge(st, 16)
                o, i_ = row_fill(0, 3)
                sync.dma_start(out=o, in_=i_).then_inc(sd, 16)
                sync.wait_ge(sd, 32)

    return nc


def kernel(x) -> np.ndarray:
    x = np.asarray(x, dtype=np.float32)
    assert x.shape == (B, C, H, W)
    if "nc" not in _cache:
        _cache["nc"] = _build()
    nc = _cache["nc"]
    in_maps = [{"x": _prep(x[i * PB : (i + 1) * PB])} for i in range(N_CORES)]
    res = run_bass_kernel_spmd(nc, in_maps, list(range(N_CORES)))
    parts = [
        np.transpose(r["out"], (1, 2, 0, 3, 4)).reshape(PB, C * F, H, W)
        for r in res.results
    ]
    return np.concatenate(parts, axis=0)


# revision 10
# speedup vs baseline: 1.0643x; 1.0643x over previous
"""ExtractOverlappingPatches Trainium2 kernel, v12.

Input  x:   (16, 64, 128, 128) f32
Output y:   (16, 576, 128, 128) f32 where
            y[b, c*9 + (i*3+j), h, w] = x[b, c, h+i-1, w+j-1] (zero padded).

Strategy (pure memory movement, target_regime=memory):
  - Shard batch 16 -> 2 per core across 8 NeuronCores; run the same
    single-core program everywhere (data-parallel over batch).
  - Device input is a guard-padded flat layout: one zero row above and below
    the (b c h) = q row stack, plus one zero guard column appended to every
    row (row pitch 129) and one leading zero element.  Every out-of-range
    column read of a shift then lands on a zero guard element, so horizontal
    boundaries come out correct straight from the copies.
  - Device output is f-major [9, PB, C, H, W]: for fixed f the output block
    is a flat image stack, so a shift is a strided flat copy.  The three
    shifts sharing an i are ONE 3-dim DMA [q=16384, f=3, w=128] whose middle
    dim walks both the output f blocks and the source column offsets
    (source step 1 elem).  All DRAM->DRAM, no SBUF staging.
  - The only remaining defects are the h=0 / h=127 boundary rows of the
    i=0 / i=2 triples (the flat q axis wraps into the neighboring image);
    ONE fused DMA [384, 2, 128] overwrites all of them with zeros from a
    const tensor.  It is ordered behind both producer triples by explicit
    completion-semaphore waits before issue -- ring FIFO alone does NOT
    order byte-level write-after-write on real queues (descriptors of
    adjacent DMAs drain on different SDMA engines).  The i=1 triple has no
    boundary defects, so it runs concurrently with the fill on the other
    ring.
  - 4 DMAs over the two HWDGE rings (SP + ACT): the i=0 and i=2 triples at
    ring depth 1, then the fill next to the i=1 triple at depth 2.  The
    SWDGE (gpsimd) ring is not used: its descriptor ring tops out below the
    49k descriptors a triple needs.
  - The block is closed without the default all-engine exit barrier: the
    sync engine's semaphore waits already observe every DMA completion, so
    it halts last with all data landed, and the barrier's release chain
    would only add idle time after the final DMA retires.
  - Host gather transposes [9, PB, C, H, W] -> [PB, C*9, H, W] per core.
"""

import dataclasses

import numpy as np

import concourse.bass as bass
import concourse.mybir as mybir
from concourse.bass_utils import run_bass_kernel_spmd

N_CORES = 8
B, C, H, W = 16, 64, 128, 128
PB = B // N_CORES  # batches per core
KH, KW = 3, 3
F = KH * KW
P = PB * C  # images per core == 128
Q = P * H  # merged (b c h) rows per core == 16384
WP = W + 1  # padded row pitch (zero guard column)
XPAD = 1 + (Q + 2) * WP  # leading zero + (guard row, q rows, guard row)

_cache = {}


def _prep(x_shard: np.ndarray) -> np.ndarray:
    """Pack one core's [PB, C, H, W] input into the guard-padded flat layout."""
    buf = np.zeros(XPAD, dtype=np.float32)
    rows = buf[1 + WP : 1 + WP * (Q + 1)].reshape(Q, WP)
    rows[:, :W] = x_shard.reshape(Q, W)
    return buf


def _build() -> bass.Bass:
    nc = bass.Bass()
    dt = mybir.dt.float32
    x = nc.dram_tensor("x", [XPAD], dt, kind="ExternalInput")
    out = nc.dram_tensor("out", [F, PB, C, H, W], dt, kind="ExternalOutput")
    zeros = nc.inline_tensor(np.zeros(2 * KW * P * W, dtype=np.float32), name="zconst")

    # buf index of x[r, c] is 1 + (r+1)*WP + c; shift f=(i,j) at out row q
    # reads r = q+i-1, c = w+j-1  ->  src offset (i*WP + j) + q*WP + w.
    def copy_group(f0, n, f_step, src_step):
        """One DMA covering shifts f0, f0+f_step, ... (n of them)."""
        i0, j0 = divmod(f0, KW)
        o = dataclasses.replace(
            out[0, 0, 0, 0, :],
            offset=f0 * Q * W,
            ap=[[W, Q], [f_step * Q * W, n], [1, W]],
        )
        i_ = dataclasses.replace(
            x[:],
            offset=i0 * WP + j0,
            ap=[[WP, Q], [src_step, n], [1, W]],
        )
        return o, i_

    def row_fill(f0, n):
        """Zero h=0 rows of shifts f0..f0+n-1 (i=0) and h=127 rows of
        f0+6..f0+6+n-1 (i=2): dims [(f b c)=n*128, h-side pair=2, w=128]."""
        o = dataclasses.replace(
            out[0, 0, 0, 0, :],
            offset=f0 * Q * W,
            ap=[[H * W, n * P], [2 * KW * Q * W + (H - 1) * W, 2], [1, W]],
        )
        z = dataclasses.replace(
            zeros[:], offset=0, ap=[[W, n * P], [n * P * W, 2], [1, W]]
        )
        return o, z

    with (
        nc.semaphore("st") as st,  # ACT ring: i=2 triple completion
        nc.semaphore("sa") as sa,  # SP ring: i=0 triple completion
        nc.semaphore("sd") as sd,  # everything else / final drain
    ):
        # Drive BassBlock manually so the block can end without the default
        # all-engine exit barrier (see docstring).
        block = bass.BassBlock(nc, f"block_{nc.next_id()}")
        nc.cur_block = block
        block.__enter__()

        # SP ring: i=0 triple, then the fused boundary-row fill (after
        # both producer triples' completion sems).  ACT ring: i=2
        # triple first (the fill waits on it), then the i=1 triple.
        @block.scalar
        def _(scalar):
            o, i_ = copy_group(6, 3, 1, 1)  # i=2 triple (f6,f7,f8)
            scalar.dma_start(out=o, in_=i_).then_inc(st, 16)
            o, i_ = copy_group(3, 3, 1, 1)  # i=1 triple (f3,f4,f5)
            scalar.dma_start(out=o, in_=i_).then_inc(sd, 16)

        @block.sync
        def _(sync):
            o, i_ = copy_group(0, 3, 1, 1)  # i=0 triple (f0,f1,f2)
            sync.dma_start(out=o, in_=i_).then_inc(sa, 16)
            sync.wait_ge(sa, 16)
            sync.wait_ge(st, 16)
            o, i_ = row_fill(0, 3)
            sync.dma_start(out=o, in_=i_).then_inc(sd, 16)
            sync.wait_ge(sd, 32)

        # Manual block exit: branch each engine to the end bb, skip the
        # exit barrier.
        for engine, last_body in block.last_body.items():
            with nc.body(last_body, parent=nc.cur_bb, allow_existing_parent=True):
                engine.br(block.end_bb)
        nc.switch_bb(block.end_bb)
        nc.cur_block = None

    return nc


def kernel(x) -> np.ndarray:
    x = np.asarray(x, dtype=np.float32)
    assert x.shape == (B, C, H, W)
    if "nc" not in _cache:
        _cache["nc"] = _build()
    nc = _cache["nc"]
    in_maps = [{"x": _prep(x[i * PB : (i + 1) * PB])} for i in range(N_CORES)]
    res = run_bass_kernel_spmd(nc, in_maps, list(range(N_CORES)))
    parts = [
        np.transpose(r["out"], (1, 2, 0, 3, 4)).reshape(PB, C * F, H, W)
        for r in res.results
    ]
    return np.concatenate(parts, axis=0)
